# revision 11
# baseline (speedup 1.0000x reference)
"""Additive-attention kernel (conv3x3 + linear bias + tanh + softmax +
weighted sum) for Trainium2, data-parallel over 8 NeuronCores.

Per core (B_local=16): the 3x3 SAME conv is an implicit GEMM — for each
of the 9 taps a shifted-window matmul accumulating into PSUM, contraction
over input channels (4 k-tiles of 128), fp32r at full PE rate. The input
is zero-padded host-side along W (64->66) so every tap reads a full even
window (fp32r ISA requires even innermost counts and 8B-aligned PSUM
windows); the H boundary is handled by clipping tap row ranges. The tanh
bias-add fuses Linear(h)+b_conv+b_h via the ACT bias operand. Attention
scores use a replicated-weight matmul so exp(e) lands broadcast on all
128 partitions, which lets the alpha-weighted reduction over L run as a
per-partition multiply+reduce on the vector engine with no
cross-partition traffic.
"""

import numpy as np

B, C, H, W = 128, 512, 8, 64
WP = W + 2  # width padded with one zero column each side
L = H * W
LP = H * WP
HID = 512
EMB = 512
NCORES = 8
BL = B // NCORES  # batches per core
KC = C // 128  # channel k-tiles
ME = EMB // 128  # output-channel m-tiles

# taps ordered center-first so the start=True matmul covers the whole
# PSUM tile (has_written must be set everywhere before clipped windows)
TAPS = [(1, 1), (0, 0), (0, 1), (0, 2), (1, 0), (1, 2), (2, 0), (2, 1), (2, 2)]


def _split_multiwaits(nc):
    # the walrus in this image accepts one sync wait/update per
    # instruction; move extras onto adjacent same-engine NOPs
    import bass_rust
    import concourse.mybir as mybir

    dma_ops = ("DMACopy", "DMATransposeAnt", "TriggeredCopy")
    for f in nc.m.functions:
        for blk in f.blocks:
            insts = list(blk.instructions)
            new = []
            changed = False
            for ins in insts:
                si = ins.sync_info
                if si is None:
                    new.append(ins)
                    continue
                if len(si.on_wait) > 1:
                    waits = list(si.on_wait)
                    for w in waits[:-1]:
                        nop = mybir.InstNoOp(
                            name=f"waitsplit-{nc.next_id()}", ins=[], outs=[]
                        )
                        nop.engine = ins.engine
                        nop.sync_info = bass_rust.SyncInfo(on_wait=[w], on_update=[])
                        new.append(nop)
                    si.on_wait = [waits[-1]]
                    changed = True
                if len(si.on_update) > 1 and ins.opcode not in dma_ops:
                    updates = list(si.on_update)
                    si.on_update = [updates[0]]
                    new.append(ins)
                    for u in updates[1:]:
                        nop = mybir.InstNoOp(
                            name=f"updsplit-{nc.next_id()}", ins=[], outs=[]
                        )
                        nop.engine = ins.engine
                        nop.sync_info = bass_rust.SyncInfo(on_wait=[], on_update=[u])
                        new.append(nop)
                    changed = True
                else:
                    new.append(ins)
            if changed:
                blk.instructions = new


def _build_nc():
    import concourse.bass as bass
    import concourse.tile as tile
    from concourse import mybir

    F = mybir.dt.float32
    R = mybir.dt.float32r
    Act = mybir.ActivationFunctionType

    nc = bass.Bass(trn_type="TRN2")

    x_d = nc.dram_tensor("x", [BL, KC, 128, LP], R, kind="ExternalInput")
    ht_d = nc.dram_tensor("ht", [KC, 128, BL], R, kind="ExternalInput")
    kw_d = nc.dram_tensor("kw", [KC, 128, 9, EMB], R, kind="ExternalInput")
    wht_d = nc.dram_tensor("wht", [KC, 128, EMB], R, kind="ExternalInput")
    wrep_d = nc.dram_tensor("wrep", [ME, 128, 128], R, kind="ExternalInput")
    beb_d = nc.dram_tensor("beb", [ME, 128], F, kind="ExternalInput")
    attT_d = nc.dram_tensor("attT", [C, BL], F, kind="ExternalOutput")
    alpha_d = nc.dram_tensor("alpha", [BL, L], F, kind="ExternalOutput")

    with tile.TileContext(nc) as tc:
        with (
            tc.tile_pool(name="const", bufs=1) as cpool,
            tc.tile_pool(name="xb", bufs=3) as xpool,
            tc.tile_pool(name="ft", bufs=8) as fpool,
            tc.tile_pool(name="eb", bufs=3) as epool,
            tc.tile_pool(name="sc", bufs=2) as scpool,
            tc.tile_pool(name="sm", bufs=4) as smpool,
            tc.tile_pool(name="px", bufs=6, space="PSUM") as pxpool,
            tc.tile_pool(name="pe", bufs=2, space="PSUM") as pepool,
        ):
            # --- constants ---
            KW = []
            for k in range(KC):
                t = cpool.tile([128, 9, EMB], R, tag=f"kw{k}")
                nc.sync.dma_start(out=t, in_=kw_d[k, :, :, :])
                KW.append(t)
            WHT = []
            for k in range(KC):
                t = cpool.tile([128, EMB], R, tag=f"wht{k}")
                nc.sync.dma_start(out=t, in_=wht_d[k, :, :])
                WHT.append(t)
            HT = cpool.tile([128, KC, BL], R, tag="ht")
            nc.sync.dma_start(out=HT, in_=ht_d[:, :, :].rearrange("k p b -> p k b"))
            WREP = cpool.tile([128, ME, 128], R, tag="wrep")
            nc.sync.dma_start(
                out=WREP, in_=wrep_d[:, :, :].rearrange("m p j -> p m j")
            )
            BEB = cpool.tile([128, ME], F, tag="beb")
            nc.sync.dma_start(out=BEB, in_=beb_d[:, :].rearrange("m p -> p m"))

            # --- g = W_h @ h^T + (b_h + b_conv), laid out [EMB-part, b] ---
            G = cpool.tile([128, ME, BL], F, tag="g")
            for m in range(ME):
                pg = pepool.tile([128, BL], F, tag="pe")
                for k in range(KC):
                    nc.tensor.matmul(
                        out=pg,
                        lhsT=WHT[k][:, m * 128 : (m + 1) * 128],
                        rhs=HT[:, k, :],
                        start=(k == 0),
                        stop=(k == KC - 1),
                    )
                nc.vector.tensor_scalar_add(
                    out=G[:, m, :], in0=pg, scalar1=BEB[:, m : m + 1]
                )

            # --- per-batch pipeline ---
            for b in range(BL):
                XP = xpool.tile([128, KC, H, WP], R, tag="xb")
                nc.sync.dma_start(
                    out=XP,
                    in_=x_d[b, :, :, :]
                    .rearrange("k p (y w) -> p k y w", w=WP),
                )

                fts = []
                for m in range(ME):
                    px = pxpool.tile([128, H, W], F, tag="px")
                    nmm = KC * 9
                    i = 0
                    for k in range(KC):
                        xk = XP[:, k, :, :]
                        for ky, kx in TAPS:
                            dy = ky - 1
                            y0o, y0i = max(0, -dy), max(0, dy)
                            ny = H - abs(dy)
                            nc.tensor.matmul(
                                out=px[:, y0o : y0o + ny, :],
                                lhsT=KW[k][:, ky * 3 + kx, m * 128 : (m + 1) * 128],
                                rhs=xk[:, y0i : y0i + ny, kx : kx + W],
                                start=(i == 0),
                                stop=(i == nmm - 1),
                                skip_group_check=True,
                            )
                            i += 1
                    ft = fpool.tile([128, H, W], R, tag="ft")
                    nc.scalar.activation(
                        out=ft, in_=px, func=Act.Tanh, bias=G[:, m, b : b + 1]
                    )
                    fts.append(ft)

                pe = pepool.tile([128, L], F, tag="pe")
                for m in range(ME):
                    nc.tensor.matmul(
                        out=pe,
                        lhsT=WREP[:, m, :],
                        rhs=fts[m][:, :, :],
                        start=(m == 0),
                        stop=(m == ME - 1),
                    )

                expb = epool.tile([128, L], F, tag="eb")
                ssum = smpool.tile([128, 1], F, tag="ss")
                nc.scalar.activation(out=expb, in_=pe, func=Act.Exp, accum_out=ssum)
                rs = smpool.tile([128, 1], F, tag="rs")
                nc.vector.reciprocal(out=rs, in_=ssum)

                al = smpool.tile([1, L], F, tag="al")
                nc.vector.tensor_scalar_mul(
                    out=al, in0=expb[0:1, :], scalar1=rs[0:1, :]
                )
                nc.sync.dma_start(out=alpha_d[b, :], in_=al)

                expb3 = expb[:, :].rearrange("p (y w) -> p y w", w=W)
                attacc = smpool.tile([128, KC], F, tag="aa")
                for k in range(KC):
                    scr = scpool.tile([128, H, W], F, tag="sc")
                    nc.vector.tensor_mul(
                        out=scr,
                        in0=XP[:, k, :, 1 : 1 + W].bitcast(F),
                        in1=expb3,
                    )
                    nc.vector.reduce_sum(
                        out=attacc[:, k : k + 1],
                        in_=scr,
                        axis=mybir.AxisListType.XY,
                    )
                attf = smpool.tile([128, KC], F, tag="af")
                nc.vector.tensor_scalar_mul(out=attf, in0=attacc, scalar1=rs)
                nc.sync.dma_start(
                    out=attT_d[:, :].rearrange("(k p) b -> p k b", p=128)[:, :, b],
                    in_=attf,
                )

    _split_multiwaits(nc)
    return nc


_last_exec_ns = None
_last_trace = None


def kernel(conv_f, h, W_h, b_h, K_conv, b_conv, w_att, b_att):
    from concourse.bass_utils import run_bass_kernel_spmd

    conv_f = np.ascontiguousarray(conv_f, dtype=np.float32)
    h = np.ascontiguousarray(h, dtype=np.float32)

    kw = np.ascontiguousarray(np.transpose(K_conv, (1, 2, 3, 0))).reshape(
        KC, 128, 9, EMB
    )
    wht = np.ascontiguousarray(np.asarray(W_h, dtype=np.float32).T).reshape(
        KC, 128, EMB
    )
    wrep = np.ascontiguousarray(
        np.broadcast_to(
            np.asarray(w_att, dtype=np.float32).reshape(ME, 128, 1), (ME, 128, 128)
        )
    )
    beb = np.ascontiguousarray(
        (np.asarray(b_conv, dtype=np.float32) + np.asarray(b_h, dtype=np.float32))
    ).reshape(ME, 128)

    x_pad = np.zeros((NCORES, BL, KC, 128, H, WP), dtype=np.float32)
    x_pad[..., 1 : 1 + W] = conv_f.reshape(NCORES, BL, KC, 128, H, W)
    x_pad = x_pad.reshape(NCORES, BL, KC, 128, LP)
    hs = h.reshape(NCORES, BL, HID)

    in_maps = []
    for i in range(NCORES):
        ht = np.ascontiguousarray(hs[i].T).reshape(KC, 128, BL)
        in_maps.append(
            {
                "x": x_pad[i],
                "ht": ht,
                "kw": kw,
                "wht": wht,
                "wrep": wrep,
                "beb": beb,
            }
        )

    nc = _build_nc()
    res = run_bass_kernel_spmd(nc, in_maps, core_ids=list(range(NCORES)))
    global _last_exec_ns, _last_trace
    _last_exec_ns = res.exec_time_ns
    _last_trace = res.instructions_and_trace

    att_out = np.empty((B, C), dtype=np.float32)
    alpha = np.empty((B, L), dtype=np.float32)
    for i in range(NCORES):
        att_out[i * BL : (i + 1) * BL] = res.results[i]["attT"].T
        alpha[i * BL : (i + 1) * BL] = res.results[i]["alpha"]
    return att_out, alpha
